# revision 18
# baseline (speedup 1.0000x reference)
import sys

if "/opt/trn_rl_repo" not in sys.path:
    sys.path.insert(0, "/opt/trn_rl_repo")

import numpy as np
import ml_dtypes

from concourse import bass, tile, bacc
from concourse.bass import mybir

F32 = mybir.dt.float32
BF16 = mybir.dt.bfloat16
I16 = mybir.dt.int16

N_CORES = 8
N_TOTAL = 32768
N_CORE = N_TOTAL // N_CORES  # 4096 rows per core
D = 1024
C = 64
K = 16
DEPTH = 4
M = 1024
ALU = mybir.AluOpType
AFT = mybir.ActivationFunctionType


def stages_for(n_rows):
    if n_rows >= 2048:
        out = [256, 768]
    else:
        out = [256, min(768, n_rows - 256)] if n_rows > 256 else []
    left = n_rows - sum(out)
    while left > 0:
        out.append(min(1024, left))
        left -= 1024
    return out


def build_program(dims, n_rows=N_CORE):
    stages = stages_for(n_rows)
    nc = bacc.Bacc()
    xp_d = nc.declare_dram_parameter("xp", [D, 3 * n_rows], BF16, isOutput=False)
    thr_d = nc.declare_dram_parameter("thrrep", [128, 15 * 512], F32, isOutput=False)
    lut_d = nc.declare_dram_parameter("lutT", [C * K, M], BF16, isOutput=False)
    kvec_d = nc.declare_dram_parameter("kvec", [128, 1], F32, isOutput=False)
    idx_d = nc.declare_dram_parameter("idx", [128, 16], I16, isOutput=False)
    id_d = nc.declare_dram_parameter("ident", [128, 128], BF16, isOutput=False)
    rm_d = nc.declare_dram_parameter("rmat", [64, 8 * 128], BF16, isOutput=False)
    out_d = nc.declare_dram_parameter("out", [n_rows, M], BF16, isOutput=True)

    with tile.TileContext(nc) as tc:
        from contextlib import ExitStack
        es = ExitStack()
        pers = es.enter_context(tc.tile_pool(name="pers", bufs=1))

        def ptile(shape, dtype, name):
            return pers.tile(shape, dtype, name=name, tag=name)

        # ---- persistent tiles ----
        lutT = ptile([128, 8, M], BF16, "lutT_sb")        # [(k*8+q), j, m]
        thrrep = ptile([128, 15, 8, 64], F32, "thr_sb")   # [p, node, t, c]
        kvec = ptile([128, 1], F32, "kvec_sb")            # k = p // 8
        idx = ptile([128, 16], I16, "idx_sb")
        ident = ptile([128, 128], BF16, "ident_sb")
        rmat = ptile([64, 8, 128], BF16, "rmat_sb")

        nc.sync.dma_start(idx[:], idx_d[:])
        nc.sync.dma_start(thrrep[:].rearrange("p a b c -> p (a b c)"), thr_d[:])
        nc.sync.dma_start(kvec[:], kvec_d[:])
        nc.sync.dma_start(ident[:], id_d[:])
        nc.sync.dma_start(rmat[:].rearrange("p a b -> p (a b)"), rm_d[:])
        for j in range(8):
            nc.scalar.dma_start(lutT[:, j, :], lut_d[j * 128:(j + 1) * 128, :])

        # descent temporaries (serial on DVE, single-buffered)
        tmps = [ptile([128, 8, 64], F32, f"tmp{i}_sb") for i in range(6)]
        b0, b1, b2, b3, tA, tB = tmps
        tC = ptile([128, 8, 64], F32, "tmpC_sb")
        tD = ptile([128, 8, 64], F32, "tmpD_sb")
        I8 = mybir.dt.int8
        b0i_t = ptile([128, 8, 64], I8, "b0i_sb")
        b1i_t = ptile([128, 8, 64], I8, "b1i_sb")

        chppool = es.enter_context(tc.tile_pool(name="chppool", bufs=3))
        chpool = es.enter_context(tc.tile_pool(name="chpool", bufs=2))
        bkpool = es.enter_context(tc.tile_pool(name="bkpool", bufs=2))
        btpool = es.enter_context(tc.tile_pool(name="btpool", bufs=2))
        etpool = es.enter_context(tc.tile_pool(name="etpool", bufs=3))
        opool = es.enter_context(tc.tile_pool(name="opool", bufs=2))
        pspool = es.enter_context(
            tc.tile_pool(name="pspool", bufs=3, space=bass.MemorySpace.PSUM)
        )
        ptpool = es.enter_context(
            tc.tile_pool(name="ptpool", bufs=2, space=bass.MemorySpace.PSUM)
        )

        TT = nc.vector.tensor_tensor

        def front(rows, off, r0):
            nt = rows // 128
            elem = 3 * rows
            # ---- indexed gather of bf16 planes straight from DRAM ----
            # chp[p, q, i]: q in [0,nt) hi of n=q*128+p, [nt,2nt) mid,
            # [2nt,3nt) lo; i = d*64 + c (d-major gather order).
            chp_full = chppool.tile([128, 24, 256], BF16, name="chp", tag="chp")
            chp = chp_full[:, 0:3 * nt, :]
            nc.gpsimd.dma_gather(
                chp,
                xp_d[:, off:off + elem],
                idx[:],
                num_idxs=256,
                num_idxs_reg=256,
                elem_size=elem,
                elem_step=3 * n_rows,
                transpose=True,
            )
            # exact fp32 = hi + mid + lo (triple-bf16 split)
            ch_full = chpool.tile([128, 8, 256], F32, name="ch", tag="ch")
            ch = ch_full[:, 0:nt, :]
            for h in (slice(0, 128), slice(128, 256)):
                TT(ch[:, :, h], chp[:, 0:nt, h], chp[:, nt:2 * nt, h], ALU.add)
                TT(ch[:, :, h], ch[:, :, h], chp[:, 2 * nt:3 * nt, h], ALU.add)

            # ---- tree descent on [128, nt, 64] contiguous level slices ----
            def xlev(dd):
                return ch[:, :, dd * 64:(dd + 1) * 64]

            def T(i):
                return thrrep[:, i, 0:nt, :]

            vb0, vb1, vb2, vb3 = (t_[:, 0:nt, :] for t_ in (b0, b1, b2, b3))
            vA, vB, vC, vD = (t_[:, 0:nt, :] for t_ in (tA, tB, tC, tD))
            b0i = b0i_t[:, 0:nt, :]
            b1i = b1i_t[:, 0:nt, :]

            TT(vb0, xlev(0), T(0), ALU.is_gt)
            nc.vector.tensor_copy(b0i, vb0)
            TT(vA, vb0, T(2), ALU.mult)
            TT(vA, vA, T(1), ALU.add)
            TT(vb1, xlev(1), vA, ALU.is_gt)
            nc.vector.tensor_copy(b1i, vb1)


            TT(vA, vb1, T(4), ALU.mult)
            TT(vA, vA, T(3), ALU.add)
            TT(vB, vb1, T(6), ALU.mult)
            TT(vB, vB, T(5), ALU.add)
            nc.vector.copy_predicated(vA, b0i, vB)
            TT(vb2, xlev(2), vA, ALU.is_gt)

            TT(vA, vb2, T(8), ALU.mult)
            TT(vA, vA, T(7), ALU.add)
            TT(vB, vb2, T(10), ALU.mult)
            TT(vB, vB, T(9), ALU.add)
            nc.vector.copy_predicated(vA, b1i, vB)
            TT(vC, vb2, T(12), ALU.mult)
            TT(vC, vC, T(11), ALU.add)
            TT(vD, vb2, T(14), ALU.mult)
            TT(vD, vD, T(13), ALU.add)
            nc.vector.copy_predicated(vC, b1i, vD)
            nc.vector.copy_predicated(vA, b0i, vC)
            TT(vb3, xlev(3), vA, ALU.is_gt)

            bucket_full = bkpool.tile([128, 8, 64], BF16, name="bucket",
                                      tag="bucket")
            bucket = bucket_full[:, 0:nt, :]
            nc.vector.scalar_tensor_tensor(vB, vb0, 2.0, vb1, ALU.mult, ALU.add)
            nc.vector.scalar_tensor_tensor(vC, vB, 2.0, vb2, ALU.mult, ALU.add)
            nc.vector.scalar_tensor_tensor(
                bucket, vC, 2.0, vb3, ALU.mult, ALU.add
            )

            # ---- transpose bucket to [c, n] via PE, evac via scalar ----
            bucketT_full = btpool.tile([64, 1024], BF16, name="bucketT",
                                       tag="bucketT")
            bucketT = bucketT_full[:, 0:rows]
            for t in range(nt):
                pst = ptpool.tile([64, 128], BF16, name="pst", tag="pst")
                nc.tensor.transpose(pst[:], bucket_full[:, t, :], ident[:])
                nc.scalar.activation(
                    bucketT[:, t * 128:(t + 1) * 128], pst[:], AFT.Copy
                )

            # ---- replicate c -> (k, q): 8 seeds + depth-2 copy tree,
            # issue split across the two HWDGE queues to halve latency ----
            ET_full = etpool.tile([128, 8, 1024], BF16, name="ET", tag="ET")
            ET = ET_full[:, :, 0:rows]
            for j in range(8):
                eng = nc.sync if j % 2 else nc.scalar
                eng.dma_start(ET[0:8, j, :], bucketT[8 * j:8 * j + 8, :])
            if rows <= 768:
                # tiny transfers: depth-1 fan-out, one completion hop
                for i, w in enumerate(range(8, 128, 8)):
                    eng = nc.sync if i % 2 else nc.scalar
                    eng.dma_start(ET[w:w + 8, :, :], ET[0:8, :, :])
            else:
                for i, w in enumerate((8, 16, 24)):
                    eng = nc.sync if i % 2 else nc.scalar
                    eng.dma_start(ET[w:w + 8, :, :], ET[0:8, :, :])
                for i, w in enumerate((32, 64, 96)):
                    eng = nc.scalar if i % 2 else nc.sync
                    eng.dma_start(ET[w:w + 32, :, :], ET[0:32, :, :])
            return ET, nt, r0

        def back(ctx):
            ET, nt, r0 = ctx
            # one-hot in place, split so early chunks unblock MMs
            nc.vector.tensor_scalar(ET[:, 0:4, :], ET[:, 0:4, :], kvec[:],
                                    None, ALU.is_equal)
            nc.vector.tensor_scalar(ET[:, 4:8, :], ET[:, 4:8, :], kvec[:],
                                    None, ALU.is_equal)

            # ---- matmul + output ----
            for t in range(nt):
                ps = pspool.tile([128, M], F32, name="ps", tag="ps")
                for j in range(8):
                    lhsT = ET[:, j, t * 128:(t + 1) * 128]
                    for mc in range(2):
                        nc.tensor.matmul(
                            ps[:, mc * 512:(mc + 1) * 512], lhsT,
                            lutT[:, j, mc * 512:(mc + 1) * 512],
                            start=(j == 0), stop=(j == 7),
                        )
                if t % 2 == 0:
                    osb = opool.tile([128, 2, M], BF16, name="osb", tag="osb")
                nc.scalar.activation(osb[:, t % 2, :], ps[:], AFT.Copy)
                if t % 2 == 1:
                    rr = r0 + (t - 1) * 128
                    nc.sync.dma_start(
                        out_d[rr:rr + 256, :].rearrange("(a p) m -> p a m", a=2),
                        osb[:],
                    )
        # software-pipelined emission: stage 0 unskewed (nothing to hide
        # behind), 1-stage skew afterwards to keep engine queues free of
        # head-of-line waits
        ctxs = []
        off = 0
        r0 = 0
        for i, rows in enumerate(stages):
            ctxs.append(front(rows, off, r0))
            off += 3 * rows
            r0 += rows
            if i == 0:
                back(ctxs[0])
            elif i >= 2:
                back(ctxs[i - 1])
        if len(ctxs) > 1:
            back(ctxs[-1])
        es.close()
    nc.finalize()
    return nc


def _split3(x):
    """Exact triple-bf16 split of fp32 (8+8+8 significand bits)."""
    hi = x.astype(ml_dtypes.bfloat16)
    r1 = x - hi.astype(np.float32)
    mid = r1.astype(ml_dtypes.bfloat16)
    lo = (r1 - mid.astype(np.float32)).astype(ml_dtypes.bfloat16)
    return hi, mid, lo


def _build_xp(xT, stages):
    """xT: [D, n] fp32 -> [D, 3n] bf16, per-stage [hi|mid|lo] blocks."""
    hi, mid, lo = _split3(xT)
    parts = []
    r0 = 0
    for rows in stages:
        sl = slice(r0, r0 + rows)
        parts += [hi[:, sl], mid[:, sl], lo[:, sl]]
        r0 += rows
    return np.ascontiguousarray(np.concatenate(parts, axis=1))


def _prep_inputs(inputMatrix, dims, thresholds, lut):
    x = np.asarray(inputMatrix, dtype=np.float32)
    dims = [int(v) for v in np.asarray(dims).ravel()]
    thr = np.asarray(thresholds, dtype=np.float32).reshape(C, K - 1)
    lut = np.asarray(lut, dtype=np.float32)
    stages = stages_for(N_CORE)

    xps = [
        _build_xp(np.ascontiguousarray(x[i * N_CORE:(i + 1) * N_CORE].T), stages)
        for i in range(N_CORES)
    ]

    # threshold table [15, C]
    tbl = np.empty((15, C), dtype=np.float32)
    tbl[0] = thr[:, 0]
    pairs = [(1, 2), (3, 4), (5, 6), (7, 8), (9, 10), (11, 12), (13, 14)]
    for i, (lo_, hi_) in enumerate(pairs):
        tbl[1 + 2 * i] = thr[:, lo_]
        tbl[2 + 2 * i] = thr[:, hi_] - thr[:, lo_]
    thrrep = np.ascontiguousarray(
        np.broadcast_to(tbl[None, :, None, :], (128, 15, 8, 64))
    ).reshape(128, 15 * 512)

    # lutT [j*128 + k*8 + q, m] = lut[m, 8j + q, k]
    lt = lut.reshape(M, 8, 8, K).transpose(1, 3, 2, 0).reshape(C * K, M)
    lutT = np.ascontiguousarray(lt.astype(ml_dtypes.bfloat16))

    kvec = (np.arange(128) // 8).astype(np.float32).reshape(128, 1)

    # gather index list, d-major: lst[d*64 + c] = dims[c*4 + d]
    lst = np.empty(256, dtype=np.int16)
    for d_ in range(4):
        for c_ in range(64):
            lst[d_ * 64 + c_] = dims[c_ * 4 + d_]
    idx16 = np.zeros((16, 16), dtype=np.int16)
    for j, u in enumerate(lst):
        idx16[j % 16, j // 16] = u
    idx = np.ascontiguousarray(np.tile(idx16, (8, 1)))

    ident = np.eye(128, dtype=ml_dtypes.bfloat16)

    # broadcast matrix: rmat[c, j, p] = (c == 8j + p%8)
    rmat = np.zeros((64, 8, 128), dtype=ml_dtypes.bfloat16)
    for j in range(8):
        for pp in range(128):
            rmat[8 * j + pp % 8, j, pp] = 1.0
    rmat = np.ascontiguousarray(rmat.reshape(64, 8 * 128))
    return xps, dims, thrrep, lutT, kvec, idx, ident, rmat


def prep_run(inputs):
    xps, dims_l, thrrep, lutT, kvec, idx, ident, rmat = _prep_inputs(
        inputs["inputMatrix"], inputs["dims"], inputs["thresholds"], inputs["lut"]
    )
    nc = build_program(dims_l)
    in_maps = [
        {
            "xp": xps[i],
            "thrrep": thrrep,
            "lutT": lutT,
            "kvec": kvec,
            "idx": idx,
            "ident": ident,
            "rmat": rmat,
        }
        for i in range(N_CORES)
    ]
    return in_maps, nc


def kernel(inputMatrix, dims, thresholds, lut, selection_matrix=None,
           tree_des_mat=None):
    from concourse.bass_utils import run_bass_kernel_spmd

    in_maps, nc = prep_run({
        "inputMatrix": inputMatrix, "dims": dims,
        "thresholds": thresholds, "lut": lut,
    })
    res = run_bass_kernel_spmd(nc, in_maps, list(range(N_CORES)))
    out = np.concatenate(
        [np.asarray(res.results[i]["out"]) for i in range(N_CORES)], axis=0
    )
    return out.astype(np.float32)


# revision 19
# speedup vs baseline: 1.1053x; 1.1053x over previous
import sys

if "/opt/trn_rl_repo" not in sys.path:
    sys.path.insert(0, "/opt/trn_rl_repo")

import numpy as np
import ml_dtypes

from concourse import bass, tile, bacc
from concourse.bass import mybir

F32 = mybir.dt.float32
BF16 = mybir.dt.bfloat16
I16 = mybir.dt.int16

N_CORES = 8
N_TOTAL = 32768
N_CORE = N_TOTAL // N_CORES  # 4096 rows per core
D = 1024
C = 64
K = 16
DEPTH = 4
M = 1024
ALU = mybir.AluOpType
AFT = mybir.ActivationFunctionType


def stages_for(n_rows):
    if n_rows >= 2048:
        out = [256, 768]
    else:
        out = [256, min(768, n_rows - 256)] if n_rows > 256 else []
    left = n_rows - sum(out)
    while left > 0:
        out.append(min(1024, left))
        left -= 1024
    return out


def build_program(dims, n_rows=N_CORE):
    stages = stages_for(n_rows)
    nc = bacc.Bacc()
    xp_d = nc.declare_dram_parameter("xp", [D, 3 * n_rows], BF16, isOutput=False)
    thr_d = nc.declare_dram_parameter("thrrep", [128, 15 * 512], F32, isOutput=False)
    lut_d = nc.declare_dram_parameter("lutT", [C * K, M], BF16, isOutput=False)
    kvec_d = nc.declare_dram_parameter("kvec", [128, 1], F32, isOutput=False)
    idx_d = nc.declare_dram_parameter("idx", [128, 16], I16, isOutput=False)
    id_d = nc.declare_dram_parameter("ident", [128, 128], BF16, isOutput=False)
    rm_d = nc.declare_dram_parameter("rmat", [64, 8 * 128], BF16, isOutput=False)
    out_d = nc.declare_dram_parameter("out", [n_rows, M], BF16, isOutput=True)

    with tile.TileContext(nc) as tc:
        from contextlib import ExitStack
        es = ExitStack()
        pers = es.enter_context(tc.tile_pool(name="pers", bufs=1))

        def ptile(shape, dtype, name):
            return pers.tile(shape, dtype, name=name, tag=name)

        # ---- persistent tiles ----
        lutT = ptile([128, 8, M], BF16, "lutT_sb")        # [(k*8+q), j, m]
        thrrep = ptile([128, 15, 8, 64], F32, "thr_sb")   # [p, node, t, c]
        kvec = ptile([128, 1], F32, "kvec_sb")            # k = p // 8
        idx = ptile([128, 16], I16, "idx_sb")
        ident = ptile([128, 128], BF16, "ident_sb")
        rmat = ptile([64, 8, 128], BF16, "rmat_sb")

        nc.sync.dma_start(idx[:], idx_d[:])
        nc.sync.dma_start(thrrep[:].rearrange("p a b c -> p (a b c)"), thr_d[:])
        nc.sync.dma_start(kvec[:], kvec_d[:])
        nc.sync.dma_start(ident[:], id_d[:])
        nc.sync.dma_start(rmat[:].rearrange("p a b -> p (a b)"), rm_d[:])
        for j in range(8):
            nc.scalar.dma_start(lutT[:, j, :], lut_d[j * 128:(j + 1) * 128, :])

        # descent temporaries (serial on DVE, single-buffered)
        tmps = [ptile([128, 8, 64], F32, f"tmp{i}_sb") for i in range(6)]
        b0, b1, b2, b3, tA, tB = tmps
        tC = ptile([128, 8, 64], F32, "tmpC_sb")
        tD = ptile([128, 8, 64], F32, "tmpD_sb")
        I8 = mybir.dt.int8
        b0i_t = ptile([128, 8, 64], I8, "b0i_sb")
        b1i_t = ptile([128, 8, 64], I8, "b1i_sb")

        chppool = es.enter_context(tc.tile_pool(name="chppool", bufs=3))
        chpool = es.enter_context(tc.tile_pool(name="chpool", bufs=2))
        bkpool = es.enter_context(tc.tile_pool(name="bkpool", bufs=2))
        btpool = es.enter_context(tc.tile_pool(name="btpool", bufs=2))
        etpool = es.enter_context(tc.tile_pool(name="etpool", bufs=3))
        opool = es.enter_context(tc.tile_pool(name="opool", bufs=2))
        pspool = es.enter_context(
            tc.tile_pool(name="pspool", bufs=3, space=bass.MemorySpace.PSUM)
        )
        ptpool = es.enter_context(
            tc.tile_pool(name="ptpool", bufs=2, space=bass.MemorySpace.PSUM)
        )

        TT = nc.vector.tensor_tensor

        def front(rows, off, r0):
            nt = rows // 128
            elem = 3 * rows
            # ---- indexed gather of bf16 planes straight from DRAM ----
            # chp[p, q, i]: q in [0,nt) hi of n=q*128+p, [nt,2nt) mid,
            # [2nt,3nt) lo; i = d*64 + c (d-major gather order).
            chp_full = chppool.tile([128, 24, 256], BF16, name="chp", tag="chp")
            chp = chp_full[:, 0:3 * nt, :]
            nc.gpsimd.dma_gather(
                chp,
                xp_d[:, off:off + elem],
                idx[:],
                num_idxs=256,
                num_idxs_reg=256,
                elem_size=elem,
                elem_step=3 * n_rows,
                transpose=True,
            )
            # exact fp32 = hi + mid + lo (triple-bf16 split)
            ch_full = chpool.tile([128, 8, 256], F32, name="ch", tag="ch")
            ch = ch_full[:, 0:nt, :]
            for h in (slice(0, 128), slice(128, 256)):
                TT(ch[:, :, h], chp[:, 0:nt, h], chp[:, nt:2 * nt, h], ALU.add)
                TT(ch[:, :, h], ch[:, :, h], chp[:, 2 * nt:3 * nt, h], ALU.add)

            # ---- tree descent on [128, nt, 64] contiguous level slices ----
            def xlev(dd):
                return ch[:, :, dd * 64:(dd + 1) * 64]

            def T(i):
                return thrrep[:, i, 0:nt, :]

            vb0, vb1, vb2, vb3 = (t_[:, 0:nt, :] for t_ in (b0, b1, b2, b3))
            vA, vB, vC, vD = (t_[:, 0:nt, :] for t_ in (tA, tB, tC, tD))
            b0i = b0i_t[:, 0:nt, :]
            b1i = b1i_t[:, 0:nt, :]

            TT(vb0, xlev(0), T(0), ALU.is_gt)
            nc.vector.tensor_copy(b0i, vb0)
            TT(vA, vb0, T(2), ALU.mult)
            TT(vA, vA, T(1), ALU.add)
            TT(vb1, xlev(1), vA, ALU.is_gt)
            nc.vector.tensor_copy(b1i, vb1)


            TT(vA, vb1, T(4), ALU.mult)
            TT(vA, vA, T(3), ALU.add)
            TT(vB, vb1, T(6), ALU.mult)
            TT(vB, vB, T(5), ALU.add)
            nc.vector.copy_predicated(vA, b0i, vB)
            TT(vb2, xlev(2), vA, ALU.is_gt)

            TT(vA, vb2, T(8), ALU.mult)
            TT(vA, vA, T(7), ALU.add)
            TT(vB, vb2, T(10), ALU.mult)
            TT(vB, vB, T(9), ALU.add)
            nc.vector.copy_predicated(vA, b1i, vB)
            TT(vC, vb2, T(12), ALU.mult)
            TT(vC, vC, T(11), ALU.add)
            TT(vD, vb2, T(14), ALU.mult)
            TT(vD, vD, T(13), ALU.add)
            nc.vector.copy_predicated(vC, b1i, vD)
            nc.vector.copy_predicated(vA, b0i, vC)
            TT(vb3, xlev(3), vA, ALU.is_gt)

            bucket_full = bkpool.tile([128, 8, 64], BF16, name="bucket",
                                      tag="bucket")
            bucket = bucket_full[:, 0:nt, :]
            nc.vector.scalar_tensor_tensor(vB, vb0, 2.0, vb1, ALU.mult, ALU.add)
            nc.vector.scalar_tensor_tensor(vC, vB, 2.0, vb2, ALU.mult, ALU.add)
            nc.vector.scalar_tensor_tensor(
                bucket, vC, 2.0, vb3, ALU.mult, ALU.add
            )

            # ---- transpose bucket to [c, n] via PE, evac via scalar ----
            bucketT_full = btpool.tile([64, 1024], BF16, name="bucketT",
                                       tag="bucketT")
            bucketT = bucketT_full[:, 0:rows]
            for t in range(nt):
                pst = ptpool.tile([64, 128], BF16, name="pst", tag="pst")
                nc.tensor.transpose(pst[:], bucket_full[:, t, :], ident[:])
                nc.scalar.activation(
                    bucketT[:, t * 128:(t + 1) * 128], pst[:], AFT.Copy
                )

            # ---- replicate c -> (k, q): 8 seeds + depth-2 copy tree,
            # issue split across the two HWDGE queues to halve latency ----
            ET_full = etpool.tile([128, 8, 1024], BF16, name="ET", tag="ET")
            ET = ET_full[:, :, 0:rows]
            for j in range(8):
                eng = nc.sync if j % 2 else nc.scalar
                eng.dma_start(ET[0:8, j, :], bucketT[8 * j:8 * j + 8, :])
            for i, w in enumerate((8, 16, 24)):
                eng = nc.sync if i % 2 else nc.scalar
                eng.dma_start(ET[w:w + 8, :, :], ET[0:8, :, :])
            for i, w in enumerate((32, 64, 96)):
                eng = nc.scalar if i % 2 else nc.sync
                eng.dma_start(ET[w:w + 32, :, :], ET[0:32, :, :])
            return ET, nt, r0

        def back(ctx):
            ET, nt, r0 = ctx
            # one-hot in place, split so early chunks unblock MMs
            nc.vector.tensor_scalar(ET[:, 0:4, :], ET[:, 0:4, :], kvec[:],
                                    None, ALU.is_equal)
            nc.vector.tensor_scalar(ET[:, 4:8, :], ET[:, 4:8, :], kvec[:],
                                    None, ALU.is_equal)

            # ---- matmul + output ----
            for t in range(nt):
                ps = pspool.tile([128, M], F32, name="ps", tag="ps")
                for j in range(8):
                    lhsT = ET[:, j, t * 128:(t + 1) * 128]
                    for mc in range(2):
                        nc.tensor.matmul(
                            ps[:, mc * 512:(mc + 1) * 512], lhsT,
                            lutT[:, j, mc * 512:(mc + 1) * 512],
                            start=(j == 0), stop=(j == 7),
                        )
                if t % 2 == 0:
                    osb = opool.tile([128, 2, M], BF16, name="osb", tag="osb")
                nc.scalar.activation(osb[:, t % 2, :], ps[:], AFT.Copy)
                if t % 2 == 1:
                    rr = r0 + (t - 1) * 128
                    nc.sync.dma_start(
                        out_d[rr:rr + 256, :].rearrange("(a p) m -> p a m", a=2),
                        osb[:],
                    )
        # software-pipelined emission: stage 0 unskewed (nothing to hide
        # behind), 1-stage skew afterwards to keep engine queues free of
        # head-of-line waits
        ctxs = []
        off = 0
        r0 = 0
        for i, rows in enumerate(stages):
            ctxs.append(front(rows, off, r0))
            off += 3 * rows
            r0 += rows
            if i == 0:
                back(ctxs[0])
            elif i >= 2:
                back(ctxs[i - 1])
        if len(ctxs) > 1:
            back(ctxs[-1])
        es.close()
    nc.finalize()
    return nc


def _split3(x):
    """Exact triple-bf16 split of fp32 (8+8+8 significand bits)."""
    hi = x.astype(ml_dtypes.bfloat16)
    r1 = x - hi.astype(np.float32)
    mid = r1.astype(ml_dtypes.bfloat16)
    lo = (r1 - mid.astype(np.float32)).astype(ml_dtypes.bfloat16)
    return hi, mid, lo


def _build_xp(xT, stages):
    """xT: [D, n] fp32 -> [D, 3n] bf16, per-stage [hi|mid|lo] blocks."""
    hi, mid, lo = _split3(xT)
    parts = []
    r0 = 0
    for rows in stages:
        sl = slice(r0, r0 + rows)
        parts += [hi[:, sl], mid[:, sl], lo[:, sl]]
        r0 += rows
    return np.ascontiguousarray(np.concatenate(parts, axis=1))


def _prep_inputs(inputMatrix, dims, thresholds, lut):
    x = np.asarray(inputMatrix, dtype=np.float32)
    dims = [int(v) for v in np.asarray(dims).ravel()]
    thr = np.asarray(thresholds, dtype=np.float32).reshape(C, K - 1)
    lut = np.asarray(lut, dtype=np.float32)
    stages = stages_for(N_CORE)

    xps = [
        _build_xp(np.ascontiguousarray(x[i * N_CORE:(i + 1) * N_CORE].T), stages)
        for i in range(N_CORES)
    ]

    # threshold table [15, C]
    tbl = np.empty((15, C), dtype=np.float32)
    tbl[0] = thr[:, 0]
    pairs = [(1, 2), (3, 4), (5, 6), (7, 8), (9, 10), (11, 12), (13, 14)]
    for i, (lo_, hi_) in enumerate(pairs):
        tbl[1 + 2 * i] = thr[:, lo_]
        tbl[2 + 2 * i] = thr[:, hi_] - thr[:, lo_]
    thrrep = np.ascontiguousarray(
        np.broadcast_to(tbl[None, :, None, :], (128, 15, 8, 64))
    ).reshape(128, 15 * 512)

    # lutT [j*128 + k*8 + q, m] = lut[m, 8j + q, k]
    lt = lut.reshape(M, 8, 8, K).transpose(1, 3, 2, 0).reshape(C * K, M)
    lutT = np.ascontiguousarray(lt.astype(ml_dtypes.bfloat16))

    kvec = (np.arange(128) // 8).astype(np.float32).reshape(128, 1)

    # gather index list, d-major: lst[d*64 + c] = dims[c*4 + d]
    lst = np.empty(256, dtype=np.int16)
    for d_ in range(4):
        for c_ in range(64):
            lst[d_ * 64 + c_] = dims[c_ * 4 + d_]
    idx16 = np.zeros((16, 16), dtype=np.int16)
    for j, u in enumerate(lst):
        idx16[j % 16, j // 16] = u
    idx = np.ascontiguousarray(np.tile(idx16, (8, 1)))

    ident = np.eye(128, dtype=ml_dtypes.bfloat16)

    # broadcast matrix: rmat[c, j, p] = (c == 8j + p%8)
    rmat = np.zeros((64, 8, 128), dtype=ml_dtypes.bfloat16)
    for j in range(8):
        for pp in range(128):
            rmat[8 * j + pp % 8, j, pp] = 1.0
    rmat = np.ascontiguousarray(rmat.reshape(64, 8 * 128))
    return xps, dims, thrrep, lutT, kvec, idx, ident, rmat


def prep_run(inputs):
    xps, dims_l, thrrep, lutT, kvec, idx, ident, rmat = _prep_inputs(
        inputs["inputMatrix"], inputs["dims"], inputs["thresholds"], inputs["lut"]
    )
    nc = build_program(dims_l)
    in_maps = [
        {
            "xp": xps[i],
            "thrrep": thrrep,
            "lutT": lutT,
            "kvec": kvec,
            "idx": idx,
            "ident": ident,
            "rmat": rmat,
        }
        for i in range(N_CORES)
    ]
    return in_maps, nc


def kernel(inputMatrix, dims, thresholds, lut, selection_matrix=None,
           tree_des_mat=None):
    from concourse.bass_utils import run_bass_kernel_spmd

    in_maps, nc = prep_run({
        "inputMatrix": inputMatrix, "dims": dims,
        "thresholds": thresholds, "lut": lut,
    })
    res = run_bass_kernel_spmd(nc, in_maps, list(range(N_CORES)))
    out = np.concatenate(
        [np.asarray(res.results[i]["out"]) for i in range(N_CORES)], axis=0
    )
    return out.astype(np.float32)


# revision 20
# speedup vs baseline: 1.1117x; 1.0058x over previous
import sys

if "/opt/trn_rl_repo" not in sys.path:
    sys.path.insert(0, "/opt/trn_rl_repo")

import numpy as np
import ml_dtypes

from concourse import bass, tile, bacc
from concourse.bass import mybir

F32 = mybir.dt.float32
BF16 = mybir.dt.bfloat16
I16 = mybir.dt.int16

N_CORES = 8
N_TOTAL = 32768
N_CORE = N_TOTAL // N_CORES  # 4096 rows per core
D = 1024
C = 64
K = 16
DEPTH = 4
M = 1024
ALU = mybir.AluOpType
AFT = mybir.ActivationFunctionType


def stages_for(n_rows):
    if n_rows >= 2048:
        out = [256, 768]
    else:
        out = [256, min(768, n_rows - 256)] if n_rows > 256 else []
    left = n_rows - sum(out)
    while left > 0:
        out.append(min(1024, left))
        left -= 1024
    return out


def build_program(dims, n_rows=N_CORE):
    stages = stages_for(n_rows)
    nc = bacc.Bacc()
    xp_d = nc.declare_dram_parameter("xp", [D, 3 * n_rows], BF16, isOutput=False)
    thr_d = nc.declare_dram_parameter("thrrep", [128, 15 * 512], F32, isOutput=False)
    lut_d = nc.declare_dram_parameter("lutT", [C * K, M], BF16, isOutput=False)
    kvec_d = nc.declare_dram_parameter("kvec", [128, 1], F32, isOutput=False)
    idx_d = nc.declare_dram_parameter("idx", [128, 16], I16, isOutput=False)
    id_d = nc.declare_dram_parameter("ident", [128, 128], BF16, isOutput=False)
    rm_d = nc.declare_dram_parameter("rmat", [64, 8 * 128], BF16, isOutput=False)
    out_d = nc.declare_dram_parameter("out", [n_rows, M], BF16, isOutput=True)

    with tile.TileContext(nc) as tc:
        from contextlib import ExitStack
        es = ExitStack()
        pers = es.enter_context(tc.tile_pool(name="pers", bufs=1))

        def ptile(shape, dtype, name):
            return pers.tile(shape, dtype, name=name, tag=name)

        # ---- persistent tiles ----
        lutT = ptile([128, 8, M], BF16, "lutT_sb")        # [(k*8+q), j, m]
        thrrep = ptile([128, 15, 8, 64], F32, "thr_sb")   # [p, node, t, c]
        kvec = ptile([128, 1], F32, "kvec_sb")            # k = p // 8
        idx = ptile([128, 16], I16, "idx_sb")
        ident = ptile([128, 128], BF16, "ident_sb")
        rmat = ptile([64, 8, 128], BF16, "rmat_sb")

        nc.sync.dma_start(idx[:], idx_d[:])
        nc.sync.dma_start(thrrep[:].rearrange("p a b c -> p (a b c)"), thr_d[:])
        nc.sync.dma_start(kvec[:], kvec_d[:])
        nc.sync.dma_start(ident[:], id_d[:])
        nc.sync.dma_start(rmat[:].rearrange("p a b -> p (a b)"), rm_d[:])
        for j in range(8):
            nc.scalar.dma_start(lutT[:, j, :], lut_d[j * 128:(j + 1) * 128, :])

        # descent temporaries (serial on DVE, single-buffered)
        tmps = [ptile([128, 8, 64], F32, f"tmp{i}_sb") for i in range(6)]
        b0, b1, b2, b3, tA, tB = tmps
        tC = ptile([128, 8, 64], F32, "tmpC_sb")
        tD = ptile([128, 8, 64], F32, "tmpD_sb")
        I8 = mybir.dt.int8
        b0i_t = ptile([128, 8, 64], I8, "b0i_sb")
        b1i_t = ptile([128, 8, 64], I8, "b1i_sb")

        chppool = es.enter_context(tc.tile_pool(name="chppool", bufs=3))
        chpool = es.enter_context(tc.tile_pool(name="chpool", bufs=2))
        bkpool = es.enter_context(tc.tile_pool(name="bkpool", bufs=2))
        btpool = es.enter_context(tc.tile_pool(name="btpool", bufs=2))
        etpool = es.enter_context(tc.tile_pool(name="etpool", bufs=3))
        opool = es.enter_context(tc.tile_pool(name="opool", bufs=2))
        pspool = es.enter_context(
            tc.tile_pool(name="pspool", bufs=3, space=bass.MemorySpace.PSUM)
        )
        ptpool = es.enter_context(
            tc.tile_pool(name="ptpool", bufs=2, space=bass.MemorySpace.PSUM)
        )

        TT = nc.vector.tensor_tensor

        def front(rows, off, r0):
            nt = rows // 128
            elem = 3 * rows
            # ---- indexed gather of bf16 planes straight from DRAM ----
            # chp[p, q, i]: q in [0,nt) hi of n=q*128+p, [nt,2nt) mid,
            # [2nt,3nt) lo; i = d*64 + c (d-major gather order).
            chp_full = chppool.tile([128, 24, 256], BF16, name="chp", tag="chp")
            chp = chp_full[:, 0:3 * nt, :]
            nc.gpsimd.dma_gather(
                chp,
                xp_d[:, off:off + elem],
                idx[:],
                num_idxs=256,
                num_idxs_reg=256,
                elem_size=elem,
                elem_step=3 * n_rows,
                transpose=True,
            )
            # exact fp32 = hi + mid + lo (triple-bf16 split)
            ch_full = chpool.tile([128, 8, 256], F32, name="ch", tag="ch")
            ch = ch_full[:, 0:nt, :]
            for h in (slice(0, 128), slice(128, 256)):
                TT(ch[:, :, h], chp[:, 0:nt, h], chp[:, nt:2 * nt, h], ALU.add)
                TT(ch[:, :, h], ch[:, :, h], chp[:, 2 * nt:3 * nt, h], ALU.add)

            # ---- tree descent on [128, nt, 64] contiguous level slices ----
            def xlev(dd):
                return ch[:, :, dd * 64:(dd + 1) * 64]

            def T(i):
                return thrrep[:, i, 0:nt, :]

            vb0, vb1, vb2, vb3 = (t_[:, 0:nt, :] for t_ in (b0, b1, b2, b3))
            vA, vB, vC, vD = (t_[:, 0:nt, :] for t_ in (tA, tB, tC, tD))
            b0i = b0i_t[:, 0:nt, :]
            b1i = b1i_t[:, 0:nt, :]

            TT(vb0, xlev(0), T(0), ALU.is_gt)
            nc.vector.tensor_copy(b0i, vb0)
            TT(vA, vb0, T(2), ALU.mult)
            TT(vA, vA, T(1), ALU.add)
            TT(vb1, xlev(1), vA, ALU.is_gt)
            nc.vector.tensor_copy(b1i, vb1)


            TT(vA, vb1, T(4), ALU.mult)
            TT(vA, vA, T(3), ALU.add)
            TT(vB, vb1, T(6), ALU.mult)
            TT(vB, vB, T(5), ALU.add)
            nc.vector.copy_predicated(vA, b0i, vB)
            TT(vb2, xlev(2), vA, ALU.is_gt)

            TT(vA, vb2, T(8), ALU.mult)
            TT(vA, vA, T(7), ALU.add)
            TT(vB, vb2, T(10), ALU.mult)
            TT(vB, vB, T(9), ALU.add)
            nc.vector.copy_predicated(vA, b1i, vB)
            TT(vC, vb2, T(12), ALU.mult)
            TT(vC, vC, T(11), ALU.add)
            TT(vD, vb2, T(14), ALU.mult)
            TT(vD, vD, T(13), ALU.add)
            nc.vector.copy_predicated(vC, b1i, vD)
            nc.vector.copy_predicated(vA, b0i, vC)
            TT(vb3, xlev(3), vA, ALU.is_gt)

            bucket_full = bkpool.tile([128, 8, 64], BF16, name="bucket",
                                      tag="bucket")
            bucket = bucket_full[:, 0:nt, :]
            nc.vector.scalar_tensor_tensor(vB, vb0, 2.0, vb1, ALU.mult, ALU.add)
            nc.vector.scalar_tensor_tensor(vC, vB, 2.0, vb2, ALU.mult, ALU.add)
            nc.vector.scalar_tensor_tensor(
                bucket, vC, 2.0, vb3, ALU.mult, ALU.add
            )

            # ---- transpose bucket to [c, n] via PE, evac via scalar ----
            bucketT_full = btpool.tile([64, 1024], BF16, name="bucketT",
                                       tag="bucketT")
            bucketT = bucketT_full[:, 0:rows]
            for t in range(nt):
                pst = ptpool.tile([64, 128], BF16, name="pst", tag="pst")
                nc.tensor.transpose(pst[:], bucket_full[:, t, :], ident[:])
                nc.scalar.activation(
                    bucketT[:, t * 128:(t + 1) * 128], pst[:], AFT.Copy
                )

            # ---- replicate c -> (k, q): 8 seeds + depth-2 copy tree,
            # issue split across the two HWDGE queues to halve latency ----
            ET_full = etpool.tile([128, 8, 1024], BF16, name="ET", tag="ET")
            ET = ET_full[:, :, 0:rows]
            if rows > 768:
                for j in range(8):
                    eng = nc.sync if j % 2 else nc.scalar
                    eng.dma_start(ET[0:8, j, :], bucketT[8 * j:8 * j + 8, :])
                for i, w in enumerate((8, 16, 24)):
                    eng = nc.sync if i % 2 else nc.scalar
                    eng.dma_start(ET[w:w + 8, :, :], ET[0:8, :, :])
                for i, w in enumerate((32, 64, 96)):
                    eng = nc.scalar if i % 2 else nc.sync
                    eng.dma_start(ET[w:w + 32, :, :], ET[0:32, :, :])
            return ET, nt, r0, bucketT, rows

        def back(ctx):
            ET, nt, r0, bucketT, rows = ctx
            if rows <= 768:
                # small lead-in stages: low-latency PE broadcast encode,
                # sharing the main PSUM ring (no extra banks)
                for j in range(8):
                    h0 = 0
                    while h0 < rows:
                        hn = min(512, rows - h0)
                        bps = pspool.tile([128, M], F32, name="ps", tag="ps")
                        nc.tensor.matmul(
                            bps[:, 0:hn], rmat[:, j, :],
                            bucketT[:, h0:h0 + hn], start=True, stop=True,
                        )
                        nc.vector.tensor_scalar(
                            ET[:, j, h0:h0 + hn], bps[:, 0:hn], kvec[:],
                            None, ALU.is_equal,
                        )
                        h0 += hn
            else:
                # one-hot in place, split so early chunks unblock MMs
                nc.vector.tensor_scalar(ET[:, 0:4, :], ET[:, 0:4, :], kvec[:],
                                        None, ALU.is_equal)
                nc.vector.tensor_scalar(ET[:, 4:8, :], ET[:, 4:8, :], kvec[:],
                                        None, ALU.is_equal)

            # ---- matmul + output ----
            for t in range(nt):
                ps = pspool.tile([128, M], F32, name="ps", tag="ps")
                for j in range(8):
                    lhsT = ET[:, j, t * 128:(t + 1) * 128]
                    for mc in range(2):
                        nc.tensor.matmul(
                            ps[:, mc * 512:(mc + 1) * 512], lhsT,
                            lutT[:, j, mc * 512:(mc + 1) * 512],
                            start=(j == 0), stop=(j == 7),
                        )
                if t % 2 == 0:
                    osb = opool.tile([128, 2, M], BF16, name="osb", tag="osb")
                nc.scalar.activation(osb[:, t % 2, :], ps[:], AFT.Copy)
                if t % 2 == 1:
                    rr = r0 + (t - 1) * 128
                    nc.sync.dma_start(
                        out_d[rr:rr + 256, :].rearrange("(a p) m -> p a m", a=2),
                        osb[:],
                    )
        # software-pipelined emission: stage 0 unskewed (nothing to hide
        # behind), 1-stage skew afterwards to keep engine queues free of
        # head-of-line waits
        ctxs = []
        off = 0
        r0 = 0
        for i, rows in enumerate(stages):
            ctxs.append(front(rows, off, r0))
            off += 3 * rows
            r0 += rows
            if i == 0:
                back(ctxs[0])
            elif i >= 2:
                back(ctxs[i - 1])
        if len(ctxs) > 1:
            back(ctxs[-1])
        es.close()
    nc.finalize()
    return nc


def _split3(x):
    """Exact triple-bf16 split of fp32 (8+8+8 significand bits)."""
    hi = x.astype(ml_dtypes.bfloat16)
    r1 = x - hi.astype(np.float32)
    mid = r1.astype(ml_dtypes.bfloat16)
    lo = (r1 - mid.astype(np.float32)).astype(ml_dtypes.bfloat16)
    return hi, mid, lo


def _build_xp(xT, stages):
    """xT: [D, n] fp32 -> [D, 3n] bf16, per-stage [hi|mid|lo] blocks."""
    hi, mid, lo = _split3(xT)
    parts = []
    r0 = 0
    for rows in stages:
        sl = slice(r0, r0 + rows)
        parts += [hi[:, sl], mid[:, sl], lo[:, sl]]
        r0 += rows
    return np.ascontiguousarray(np.concatenate(parts, axis=1))


def _prep_inputs(inputMatrix, dims, thresholds, lut):
    x = np.asarray(inputMatrix, dtype=np.float32)
    dims = [int(v) for v in np.asarray(dims).ravel()]
    thr = np.asarray(thresholds, dtype=np.float32).reshape(C, K - 1)
    lut = np.asarray(lut, dtype=np.float32)
    stages = stages_for(N_CORE)

    xps = [
        _build_xp(np.ascontiguousarray(x[i * N_CORE:(i + 1) * N_CORE].T), stages)
        for i in range(N_CORES)
    ]

    # threshold table [15, C]
    tbl = np.empty((15, C), dtype=np.float32)
    tbl[0] = thr[:, 0]
    pairs = [(1, 2), (3, 4), (5, 6), (7, 8), (9, 10), (11, 12), (13, 14)]
    for i, (lo_, hi_) in enumerate(pairs):
        tbl[1 + 2 * i] = thr[:, lo_]
        tbl[2 + 2 * i] = thr[:, hi_] - thr[:, lo_]
    thrrep = np.ascontiguousarray(
        np.broadcast_to(tbl[None, :, None, :], (128, 15, 8, 64))
    ).reshape(128, 15 * 512)

    # lutT [j*128 + k*8 + q, m] = lut[m, 8j + q, k]
    lt = lut.reshape(M, 8, 8, K).transpose(1, 3, 2, 0).reshape(C * K, M)
    lutT = np.ascontiguousarray(lt.astype(ml_dtypes.bfloat16))

    kvec = (np.arange(128) // 8).astype(np.float32).reshape(128, 1)

    # gather index list, d-major: lst[d*64 + c] = dims[c*4 + d]
    lst = np.empty(256, dtype=np.int16)
    for d_ in range(4):
        for c_ in range(64):
            lst[d_ * 64 + c_] = dims[c_ * 4 + d_]
    idx16 = np.zeros((16, 16), dtype=np.int16)
    for j, u in enumerate(lst):
        idx16[j % 16, j // 16] = u
    idx = np.ascontiguousarray(np.tile(idx16, (8, 1)))

    ident = np.eye(128, dtype=ml_dtypes.bfloat16)

    # broadcast matrix: rmat[c, j, p] = (c == 8j + p%8)
    rmat = np.zeros((64, 8, 128), dtype=ml_dtypes.bfloat16)
    for j in range(8):
        for pp in range(128):
            rmat[8 * j + pp % 8, j, pp] = 1.0
    rmat = np.ascontiguousarray(rmat.reshape(64, 8 * 128))
    return xps, dims, thrrep, lutT, kvec, idx, ident, rmat


def prep_run(inputs):
    xps, dims_l, thrrep, lutT, kvec, idx, ident, rmat = _prep_inputs(
        inputs["inputMatrix"], inputs["dims"], inputs["thresholds"], inputs["lut"]
    )
    nc = build_program(dims_l)
    in_maps = [
        {
            "xp": xps[i],
            "thrrep": thrrep,
            "lutT": lutT,
            "kvec": kvec,
            "idx": idx,
            "ident": ident,
            "rmat": rmat,
        }
        for i in range(N_CORES)
    ]
    return in_maps, nc


def kernel(inputMatrix, dims, thresholds, lut, selection_matrix=None,
           tree_des_mat=None):
    from concourse.bass_utils import run_bass_kernel_spmd

    in_maps, nc = prep_run({
        "inputMatrix": inputMatrix, "dims": dims,
        "thresholds": thresholds, "lut": lut,
    })
    res = run_bass_kernel_spmd(nc, in_maps, list(range(N_CORES)))
    out = np.concatenate(
        [np.asarray(res.results[i]["out"]) for i in range(N_CORES)], axis=0
    )
    return out.astype(np.float32)
